# revision 1
# baseline (speedup 1.0000x reference)
"""Type-2 NUFFT (image -> non-uniform k-space) on 8 Trainium2 NeuronCores.

kspace[b,m] = sum_{x,y} image[b,x,y] * exp(-i*(kx_m*(x-128) + ky_m*(y-128)))

Decomposition per core (M sharded 8 ways, 2048 points/core):
  Cx[m,x]=cos(kx_m x'), Sx=sin(kx_m x'), Cy[m,y]=cos(ky_m y'), Sy=sin(ky_m y')
  A[m,y] = sum_x img[x,y] Cx[m,x]   (PE matmul, lhsT = CxT[x,m] table)
  Bv[m,y] = sum_x img[x,y] Sx[m,x]
  Re[m] = sum_y A*Cy - Bv*Sy        (DVE fused mul+reduce over [A|Bv]·[Cy|-Sy])
  Im[m] = -(sum_y A*Sy + Bv*Cy)     (over [A|Bv]·[-Sy|-Cy])

Trig via ScalarE Sin LUT (valid domain [-pi,pi]) with range reduction in
"turns": P = k*x'/(2pi); f = P - round(P) via the fp32 magic-constant trick;
sin = Sin(2pi*f); cos = 1 - 2*Sin(pi*f)^2.
"""

import sys

if '/opt/trn_rl_repo' not in sys.path:
    sys.path.insert(0, '/opt/trn_rl_repo')

import numpy as np

B, NX, NY, M, NCORES = 2, 256, 256, 16384, 8
ML = M // NCORES            # 2048 m-points per core
NT = ML // 128              # 16 m-tiles per core
TWO_PI = float(2.0 * np.pi)
PI = float(np.pi)
MAGIC = 12582912.0          # 1.5 * 2**23: (x + MAGIC) - MAGIC == round(x) in fp32

_CACHE = {}


def _build():
    import concourse.bacc as bacc
    import concourse.mybir as mybir
    from concourse.tile import TileContext

    A = mybir.AluOpType
    F = mybir.ActivationFunctionType
    f32 = mybir.dt.float32

    nc = bacc.Bacc("TRN2", target_bir_lowering=False, debug=False)

    image = nc.dram_tensor("image", [B, NX, NY], f32, kind="ExternalInput")
    traj = nc.dram_tensor("traj", [2, ML], f32, kind="ExternalInput")
    # xs2pi: (arange(256)-128)/(2pi) broadcast-ready row; ysb: same, replicated 128x
    xs2pi = nc.dram_tensor("xs2pi", [1, NX], f32, kind="ExternalInput")
    ysb = nc.dram_tensor("ysb", [128, NY], f32, kind="ExternalInput")
    out = nc.dram_tensor("out", [128, 4 * NT], f32, kind="ExternalOutput")

    with TileContext(nc) as tc:
        with tc.tile_pool(name="const", bufs=1) as cpool, \
             tc.tile_pool(name="xtab", bufs=1) as xpool, \
             tc.tile_pool(name="xscratch", bufs=2) as xs_pool, \
             tc.tile_pool(name="ytab", bufs=3) as ypool, \
             tc.tile_pool(name="work", bufs=3) as wpool, \
             tc.tile_pool(name="psP", bufs=1, space="PSUM") as psP, \
             tc.tile_pool(name="psAB", bufs=4, space="PSUM") as psAB:

            # ---------------- constants / inputs ----------------
            xs_sb = cpool.tile([1, NX], f32)
            nc.sync.dma_start(xs_sb[:, :], xs2pi[:, :])
            ysb_sb = cpool.tile([128, NY], f32)
            nc.sync.dma_start(ysb_sb[:, :], ysb[:, :])
            kx_row = cpool.tile([1, ML], f32)
            nc.sync.dma_start(kx_row[:, :], traj[0:1, :])
            # ky arranged one-per-partition: ky_col[p, t] = ky[t*128 + p]
            ky_col = cpool.tile([128, NT], f32)
            nc.sync.dma_start(
                ky_col[:, :], traj[1:2, :].rearrange("o (t p) -> (o p) t", p=128))

            img_sb = {}
            for b in range(B):
                for k in range(2):
                    t_ = cpool.tile([128, NY], f32, name=f"img_{b}_{k}")
                    nc.sync.dma_start(
                        t_[:, :], image[b, k * 128:(k + 1) * 128, :])
                    img_sb[(b, k)] = t_

            # ---------------- x tables: CxT/SxT [x(2x128), m(2048)] ----------
            # P[x,m] = xs2pi[x] * kx[m]  (outer product on PE, k=1, fp32)
            cxT = [xpool.tile([128, ML], f32, name=f"cxT{h}") for h in range(2)]
            sxT = [xpool.tile([128, ML], f32, name=f"sxT{h}") for h in range(2)]
            for h in range(2):
                P = psP.tile([128, ML], f32, tag="Px")
                for j in range(ML // 512):
                    nc.tensor.matmul(
                        P[:, j * 512:(j + 1) * 512],
                        xs_sb[:, h * 128:(h + 1) * 128],
                        kx_row[:, j * 512:(j + 1) * 512],
                        start=True, stop=True)
                rs = xs_pool.tile([128, ML], f32, tag="xrs")
                nc.vector.tensor_scalar(
                    rs[:, :], P[:, :], scalar1=MAGIC, scalar2=MAGIC,
                    op0=A.add, op1=A.subtract)
                fs = xs_pool.tile([128, ML], f32, tag="xfs")
                nc.vector.scalar_tensor_tensor(
                    fs[:, :], P[:, :], 1.0, rs[:, :],
                    op0=A.mult, op1=A.subtract)
                # SxT = sin(2pi f)
                nc.scalar.activation(sxT[h][:, :], fs[:, :], F.Sin, scale=TWO_PI)
                # CxT = 1 - 2*sin(pi f)^2
                sh = xs_pool.tile([128, ML], f32, tag="xsh")
                nc.scalar.activation(sh[:, :], fs[:, :], F.Sin, scale=PI)
                sq = xs_pool.tile([128, ML], f32, tag="xsq")
                nc.scalar.activation(sq[:, :], sh[:, :], F.Square)
                nc.vector.tensor_scalar(
                    cxT[h][:, :], sq[:, :], scalar1=-2.0, scalar2=1.0,
                    op0=A.mult, op1=A.add)

            # ---------------- per m-tile main loop ----------------
            out_sb = cpool.tile([128, 4 * NT], f32)
            for t in range(NT):
                # --- y tables W_re=[Cy|-Sy], W_im=[-Sy|-Cy]  [128, 512] ---
                u = ky_col[:, t:t + 1]
                p_y = ypool.tile([128, NY], f32, tag="py")
                nc.vector.tensor_scalar(
                    p_y[:, :], ysb_sb[:, :], scalar1=u, scalar2=None, op0=A.mult)
                rs_y = ypool.tile([128, NY], f32, tag="yrs")
                nc.vector.tensor_scalar(
                    rs_y[:, :], p_y[:, :], scalar1=MAGIC, scalar2=MAGIC,
                    op0=A.add, op1=A.subtract)
                fs_y = ypool.tile([128, NY], f32, tag="yfs")
                nc.vector.scalar_tensor_tensor(
                    fs_y[:, :], p_y[:, :], 1.0, rs_y[:, :],
                    op0=A.mult, op1=A.subtract)
                w_re = ypool.tile([128, 2 * NY], f32, tag="wre")
                w_im = ypool.tile([128, 2 * NY], f32, tag="wim")
                # -Sy into both slots
                nc.scalar.activation(
                    w_re[:, NY:2 * NY], fs_y[:, :], F.Sin, scale=-TWO_PI)
                nc.scalar.activation(
                    w_im[:, 0:NY], fs_y[:, :], F.Sin, scale=-TWO_PI)
                sh_y = ypool.tile([128, NY], f32, tag="ysh")
                nc.scalar.activation(sh_y[:, :], fs_y[:, :], F.Sin, scale=PI)
                sq_y = ypool.tile([128, NY], f32, tag="ysq")
                nc.scalar.activation(sq_y[:, :], sh_y[:, :], F.Square)
                nc.vector.tensor_scalar(
                    w_re[:, 0:NY], sq_y[:, :], scalar1=-2.0, scalar2=1.0,
                    op0=A.mult, op1=A.add)
                nc.vector.tensor_scalar(
                    w_im[:, NY:2 * NY], sq_y[:, :], scalar1=2.0, scalar2=-1.0,
                    op0=A.mult, op1=A.add)

                for b in range(B):
                    # --- stage 1: AB = [A | Bv]  [128, 512] in PSUM ---
                    ab = psAB.tile([128, 2 * NY], f32, tag="ab")
                    for k in range(2):
                        nc.tensor.matmul(
                            ab[:, 0:NY],
                            cxT[k][:, t * 128:(t + 1) * 128],
                            img_sb[(b, k)][:, :],
                            start=(k == 0), stop=(k == 1))
                    for k in range(2):
                        nc.tensor.matmul(
                            ab[:, NY:2 * NY],
                            sxT[k][:, t * 128:(t + 1) * 128],
                            img_sb[(b, k)][:, :],
                            start=(k == 0), stop=(k == 1))
                    # --- stage 2: fused multiply + row-reduce ---
                    scr_re = wpool.tile([128, 2 * NY], f32, tag="scr_re")
                    scr_im = wpool.tile([128, 2 * NY], f32, tag="scr_im")
                    nc.vector.scalar_tensor_tensor(
                        scr_re[:, :], ab[:, :], 1.0, w_re[:, :],
                        op0=A.mult, op1=A.mult,
                        accum_out=out_sb[:, (2 * b) * NT + t:(2 * b) * NT + t + 1])
                    nc.vector.scalar_tensor_tensor(
                        scr_im[:, :], ab[:, :], 1.0, w_im[:, :],
                        op0=A.mult, op1=A.mult,
                        accum_out=out_sb[:, (2 * b + 1) * NT + t:(2 * b + 1) * NT + t + 1])

            nc.sync.dma_start(out[:, :], out_sb[:, :])

    nc.compile()
    return nc


def kernel(image, trajectory):
    from concourse.bass_utils import run_bass_kernel_spmd

    if 'nc' not in _CACHE:
        _CACHE['nc'] = _build()
    nc = _CACHE['nc']

    image = np.ascontiguousarray(np.asarray(image, dtype=np.float32))
    trajectory = np.ascontiguousarray(np.asarray(trajectory, dtype=np.float32))

    xs = ((np.arange(NX) - NX // 2).astype(np.float64) / (2.0 * np.pi))
    xs2pi = xs.astype(np.float32).reshape(1, NX)
    ysb = np.broadcast_to(xs2pi, (128, NY)).copy()

    in_maps = []
    for c in range(NCORES):
        in_maps.append({
            "image": image,
            "traj": np.ascontiguousarray(trajectory[:, c * ML:(c + 1) * ML]),
            "xs2pi": xs2pi,
            "ysb": ysb,
        })

    res = run_bass_kernel_spmd(nc, in_maps, core_ids=list(range(NCORES)))

    kspace = np.empty((B, M), dtype=np.complex64)
    for c in range(NCORES):
        o = res.results[c]["out"]          # [128, 4*NT]
        o = o.reshape(128, 2, 2, NT)       # [p, b, reim, t]
        for b in range(B):
            re = o[:, b, 0, :].T.reshape(ML)   # m = t*128 + p
            im = o[:, b, 1, :].T.reshape(ML)
            kspace[b, c * ML:(c + 1) * ML] = re + 1j * im
    return kspace
